# revision 24
# baseline (speedup 1.0000x reference)
"""Trainium2 Bass kernel for ContrastiveMaskedPatchSimilarity loss.

Computes: per-position cosine similarity along the channel axis of two
[32, 256, 64, 64] f32 tensors, then a masked mean -> scalar.

Strategy (pure data parallel over 8 NeuronCores, batch-sharded 4 each):
  - The masked mean only needs sim at mask==1 positions (~50%). The host
    gathers just those channel columns into a packed [256, 8704] array
    per core (zero-padded), halving the HBM traffic that is this
    memory-bound problem's entire roofline. A dense-layout NEFF is
    compiled lazily as a fallback if a mask ever exceeds the packed
    capacity.
  - The kernel streams the packed u/m and produces the three
    per-position channel sums (num=sum(u*m), uu=sum(u*u), mm=sum(m*m));
    the tiny nonlinear tail (sim=num/sqrt(uu*mm), masked mean) runs on
    host. The device side is a pure streaming pipeline with no
    mid-stream epilogue stalls.
  - Layout on chip: [channel-chunk (128) = partitions, position = free].
    All input DMAs on the sync/SP HWDGE ring: 8KB/partition descriptors
    saturate the 16 SDMA engines' ~27GiB/s line rate (~430GB/s).
  - Elementwise products (u*m, u*u, m*m) on DVE/ACT, written as bf16.
  - Channel reduction via TensorE: per position-block column, the two
    chunks' product slices [128ch x 128pos] are matmul'd against
    ones[128,1] back-to-back into the same PSUM slot (start/stop
    accumulation) -> no chunk-combining ops anywhere.
  - Stats stream out per unit (DVE copy PSUM->SBUF + SWDGE DMA, emitted
    one unit late); the final unit is processed in small chunks so the
    post-last-DMA drain is ~4us.
"""

import sys
from contextlib import ExitStack

import numpy as np

sys.path.insert(0, "/opt/trn_rl_repo")

import ml_dtypes  # noqa: E402
import concourse.bass as bass  # noqa: E402
import concourse.tile as tile  # noqa: E402
from concourse import bacc, mybir  # noqa: E402
from concourse.bass_utils import run_bass_kernel_spmd  # noqa: E402

B, C, H, W = 32, 256, 64, 64
NCORES = 8
BL = B // NCORES  # batches per core: 4
HWX = H * W  # 4096
NPOS = BL * HWX  # positions per core: 16384
NCHUNK = C // 128  # channel chunks: 2

F32 = mybir.dt.float32
BF16 = mybir.dt.bfloat16

# packed capacity: max masked positions per core the fast path handles.
# counts are ~binomial(16384, 0.5) (sigma 64); 8320 is mean + 2 sigma and
# 96 above the actual per-core max for the reference's fixed seed; any
# larger mask falls back to the dense-layout NEFF.
CAPB = 65  # capacity in 128-position blocks
CAP = CAPB * 128  # 8320
DENSEB = NPOS // 128  # 128 blocks for the dense fallback

UB = 24  # blocks per main streaming unit
FINAL_CHUNKS = [4, 2, 2]  # block split of the final unit (short drain)
FB = sum(FINAL_CHUNKS)  # 8
PREFETCH = 2  # units of DMA issued ahead of compute

_CACHED_NC = {}


def build_nc(nblocks):
    ncols = nblocks * 128
    nc = bacc.Bacc(
        "TRN2", target_bir_lowering=False, debug=False, num_devices=NCORES
    )
    u_d = nc.dram_tensor("u", [C, ncols], BF16, kind="ExternalInput")
    m_d = nc.dram_tensor("m", [C, ncols], BF16, kind="ExternalInput")
    # out[p, blk, s] = stat s (num/uu/mm) of packed position blk*128+p
    out_d = nc.dram_tensor("out", [128, nblocks * 3], F32, kind="ExternalOutput")

    # unit list: spans of blocks; final FB blocks split into small chunks
    spans = []
    blk = 0
    while blk < nblocks - FB:
        w = min(UB, nblocks - FB - blk)
        spans.append((blk, w))
        blk += w
    n_main = len(spans)
    for w in FINAL_CHUNKS:
        spans.append((blk, w))
        blk += w
    assert blk == nblocks

    with tile.TileContext(nc) as tc, ExitStack() as ctx:
        const_pool = ctx.enter_context(tc.tile_pool(name="const", bufs=1))
        in_pool = ctx.enter_context(tc.tile_pool(name="inp", bufs=3))
        tmp_pool = ctx.enter_context(tc.tile_pool(name="tmp", bufs=2))
        out_pool = ctx.enter_context(tc.tile_pool(name="outp", bufs=1))
        psum_pool = ctx.enter_context(
            tc.tile_pool(name="psum", bufs=1, space="PSUM")
        )

        ones_t = const_pool.tile([128, 1], BF16)
        nc.vector.memset(ones_t[:], 1.0)
        # single PSUM bank holds all stats: cols blk*3 + s
        P = psum_pool.tile([128, nblocks, 3], F32, name="P", tag="P")
        stats_t = out_pool.tile([128, nblocks, 3], F32)

        in_tiles = {}

        def issue_dma(i):
            blk0, w = spans[i]
            small = i >= n_main
            csl = slice(blk0 * 128, (blk0 + w) * 128)
            ums = []
            for ch in range(NCHUNK):
                row0 = ch * 128
                rsl = slice(row0, row0 + 128)
                if small:
                    u_t = in_pool.tile([128, 512], BF16, name=f"qu{ch}", tag=f"qu{ch}", bufs=2)
                    m_t = in_pool.tile([128, 512], BF16, name=f"qm{ch}", tag=f"qm{ch}", bufs=2)
                    u_t, m_t = u_t[:, : w * 128], m_t[:, : w * 128]
                else:
                    u_t = in_pool.tile([128, UB * 128], BF16, name=f"u{ch}", tag=f"u{ch}")
                    m_t = in_pool.tile([128, UB * 128], BF16, name=f"m{ch}", tag=f"m{ch}")
                    u_t, m_t = u_t[:, : w * 128], m_t[:, : w * 128]
                nc.sync.dma_start(u_t, u_d[rsl, csl])
                nc.sync.dma_start(m_t, m_d[rsl, csl])
                ums.append((u_t, m_t))
            in_tiles[i] = ums

        def flush(blk0, blk1, last=False):
            # PSUM has no DMA route: DVE copy to SBUF + DMA out. Mid-
            # stream flushes ride the idle SWDGE (gpsimd) queue so their
            # waits never block the input-DMA ring; the final flush uses
            # the (by then empty) sync ring for its lower latency.
            nc.vector.tensor_copy(
                stats_t[:, blk0:blk1, :], P[:, blk0:blk1, :]
            )
            eng = nc.sync if last else nc.gpsimd
            eng.dma_start(
                out_d[:, blk0 * 3 : blk1 * 3], stats_t[:, blk0:blk1, :]
            )

        for j in range(PREFETCH):
            issue_dma(j)

        mm_ctr = 0
        for i, (blk0, w) in enumerate(spans):
            if i + PREFETCH < len(spans):
                issue_dma(i + PREFETCH)
            ums = in_tiles.pop(i)
            small = i >= n_main
            wc = w * 128

            prods = []  # prods[ch] = (num, uu, mm)
            for ch, (u_t, m_t) in enumerate(ums):
                if small:
                    num_t = tmp_pool.tile([128, 512], BF16, name=f"qnum{ch}", tag=f"qnum{ch}", bufs=2
                    )[:, :wc]
                    uu_t = tmp_pool.tile([128, 512], BF16, name=f"quu{ch}", tag=f"quu{ch}", bufs=2
                    )[:, :wc]
                    mm_t = tmp_pool.tile([128, 512], BF16, name=f"qmm{ch}", tag=f"qmm{ch}", bufs=2
                    )[:, :wc]
                else:
                    num_t = tmp_pool.tile([128, UB * 128], BF16, name=f"num{ch}", tag=f"num{ch}"
                    )[:, :wc]
                    uu_t = tmp_pool.tile([128, UB * 128], BF16, name=f"uu{ch}", tag=f"uu{ch}"
                    )[:, :wc]
                    mm_t = tmp_pool.tile([128, UB * 128], BF16, name=f"mm{ch}", tag=f"mm{ch}"
                    )[:, :wc]
                nc.vector.tensor_mul(num_t, u_t, m_t)
                nc.scalar.square(uu_t, u_t)
                # bf16 gives DVE 2x throughput: it takes num and most m*m;
                # ACT keeps the u squares and the otherwise-idle GpSimd
                # absorbs every 4th m*m so no engine exceeds ~82% of the
                # DMA cadence
                if mm_ctr % 4 == 2 and not small:
                    nc.gpsimd.tensor_mul(mm_t, m_t, m_t)
                else:
                    nc.vector.tensor_mul(mm_t, m_t, m_t)
                mm_ctr += 1
                prods.append((num_t, uu_t, mm_t))

            for s in range(3):
                for pb in range(w):
                    for ch in range(NCHUNK):
                        nc.tensor.matmul(
                            P[:, blk0 + pb, s : s + 1],
                            prods[ch][s][:, pb * 128 : (pb + 1) * 128],
                            ones_t[:, :],
                            start=(ch == 0),
                            stop=(ch == NCHUNK - 1),
                        )

            # flush the previous unit's stats one unit after its last
            # matmul was emitted, so the copy never stalls DVE
            if 0 < i:
                pblk0, pw = spans[i - 1]
                flush(pblk0, pblk0 + pw)

        blk0, w = spans[-1]
        flush(blk0, blk0 + w, last=True)

    nc.compile()
    return nc


def get_nc(nblocks=CAPB):
    if nblocks not in _CACHED_NC:
        _CACHED_NC[nblocks] = build_nc(nblocks)
    return _CACHED_NC[nblocks]


def _pack_core(u4, m4, mask4, ncols):
    """Gather masked channel-columns of 4 batches into [C, ncols] bf16."""
    usegs, msegs = [], []
    for b in range(BL):
        idx = np.nonzero(mask4[b])[0]
        usegs.append(u4[b].reshape(C, HWX)[:, idx])
        msegs.append(m4[b].reshape(C, HWX)[:, idx])
    u_p = np.concatenate(usegs, axis=1)
    cnt = u_p.shape[1]
    up = np.zeros((C, ncols), dtype=ml_dtypes.bfloat16)
    mp = np.zeros((C, ncols), dtype=ml_dtypes.bfloat16)
    up[:, :cnt] = u_p.astype(ml_dtypes.bfloat16)
    mp[:, :cnt] = np.concatenate(msegs, axis=1).astype(ml_dtypes.bfloat16)
    return up, mp, cnt


def _run(unmasked, masked, latent_mask):
    mask = np.asarray(latent_mask) != 0
    mask_flat = mask.reshape(B, HWX)
    counts = [
        int(mask_flat[i * BL : (i + 1) * BL].sum()) for i in range(NCORES)
    ]

    if max(counts) <= CAP:
        nblocks = CAPB
        in_maps, valid = [], []
        for i in range(NCORES):
            sl = slice(i * BL, (i + 1) * BL)
            up, mp, cnt = _pack_core(
                unmasked[sl], masked[sl], mask_flat[sl], CAP
            )
            in_maps.append({"u": up, "m": mp})
            w = np.zeros(CAP, dtype=bool)
            w[:cnt] = True
            valid.append(w)
    else:
        # dense fallback: all positions, mask applied on host
        nblocks = DENSEB
        in_maps, valid = [], []
        for i in range(NCORES):
            sl = slice(i * BL, (i + 1) * BL)
            up = np.ascontiguousarray(
                np.asarray(unmasked[sl], dtype=np.float32).transpose(1, 0, 2, 3)
            ).reshape(C, NPOS).astype(ml_dtypes.bfloat16)
            mp = np.ascontiguousarray(
                np.asarray(masked[sl], dtype=np.float32).transpose(1, 0, 2, 3)
            ).reshape(C, NPOS).astype(ml_dtypes.bfloat16)
            in_maps.append({"u": up, "m": mp})
            valid.append(mask_flat[sl].reshape(NPOS))

    nc = get_nc(nblocks)
    return nc, in_maps, valid, float(mask.sum()), nblocks


def _finalize(results, valid, den, nblocks):
    num = 0.0
    for res, w in zip(results, valid):
        out = np.asarray(res["out"], dtype=np.float64).reshape(
            128, nblocks, 3
        )
        # out[p, blk, s] -> stats[s, blk*128+p]
        stats = out.transpose(2, 1, 0).reshape(3, nblocks * 128)
        n, uu, mm = stats[0][w], stats[1][w], stats[2][w]
        num += (n / np.sqrt(uu * mm)).sum()
    return np.float32(num / den)


def kernel(unmasked_latent_tensors, masked_latent_tensors, latent_mask, **kw):
    nc, in_maps, valid, den, nblocks = _run(
        np.asarray(unmasked_latent_tensors, dtype=np.float32),
        np.asarray(masked_latent_tensors, dtype=np.float32),
        np.asarray(latent_mask),
    )
    res = run_bass_kernel_spmd(nc, in_maps, list(range(NCORES)))
    return _finalize(res.results, valid, den, nblocks)


def kernel_traced(unmasked_latent_tensors, masked_latent_tensors, latent_mask):
    """Like kernel() but with NTFF tracing; returns (value, BassKernelResults)."""
    nc, in_maps, valid, den, nblocks = _run(
        np.asarray(unmasked_latent_tensors, dtype=np.float32),
        np.asarray(masked_latent_tensors, dtype=np.float32),
        np.asarray(latent_mask),
    )
    res = run_bass_kernel_spmd(nc, in_maps, list(range(NCORES)), trace=True)
    return _finalize(res.results, valid, den, nblocks), res


# revision 25
# speedup vs baseline: 1.0989x; 1.0989x over previous
"""Trainium2 Bass kernel for ContrastiveMaskedPatchSimilarity loss.

Computes: per-position cosine similarity along the channel axis of two
[32, 256, 64, 64] f32 tensors, then a masked mean -> scalar.

Strategy (pure data parallel over 8 NeuronCores, batch-sharded 4 each):
  - The masked mean only needs sim at mask==1 positions (~50%). The host
    gathers just those channel columns into a packed [256, 8704] array
    per core (zero-padded), halving the HBM traffic that is this
    memory-bound problem's entire roofline. A dense-layout NEFF is
    compiled lazily as a fallback if a mask ever exceeds the packed
    capacity.
  - The kernel streams the packed u/m and produces the three
    per-position channel sums (num=sum(u*m), uu=sum(u*u), mm=sum(m*m));
    the tiny nonlinear tail (sim=num/sqrt(uu*mm), masked mean) runs on
    host. The device side is a pure streaming pipeline with no
    mid-stream epilogue stalls.
  - Layout on chip: [channel-chunk (128) = partitions, position = free].
    All input DMAs on the sync/SP HWDGE ring: 8KB/partition descriptors
    saturate the 16 SDMA engines' ~27GiB/s line rate (~430GB/s).
  - Elementwise products (u*m, u*u, m*m) on DVE/ACT, written as bf16.
  - Channel reduction via TensorE: per position-block column, the two
    chunks' product slices [128ch x 128pos] are matmul'd against
    ones[128,1] back-to-back into the same PSUM slot (start/stop
    accumulation) -> no chunk-combining ops anywhere.
  - Stats stream out per unit (DVE copy PSUM->SBUF + SWDGE DMA, emitted
    one unit late); the final unit is processed in small chunks so the
    post-last-DMA drain is ~4us.
"""

import sys
from contextlib import ExitStack

import numpy as np

sys.path.insert(0, "/opt/trn_rl_repo")

import ml_dtypes  # noqa: E402
import concourse.bass as bass  # noqa: E402
import concourse.tile as tile  # noqa: E402
from concourse import bacc, mybir  # noqa: E402
from concourse.bass_utils import run_bass_kernel_spmd  # noqa: E402

B, C, H, W = 32, 256, 64, 64
NCORES = 8
BL = B // NCORES  # batches per core: 4
HWX = H * W  # 4096
NPOS = BL * HWX  # positions per core: 16384
NCHUNK = C // 128  # channel chunks: 2

F32 = mybir.dt.float32
BF16 = mybir.dt.bfloat16

# packed capacity: max masked positions per core the fast path handles.
# counts are ~binomial(16384, 0.5) (sigma 64); 8320 is mean + 2 sigma and
# 96 above the actual per-core max for the reference's fixed seed; any
# larger mask falls back to the dense-layout NEFF.
CAPB = 65  # capacity in 128-position blocks
CAP = CAPB * 128  # 8320
DENSEB = NPOS // 128  # 128 blocks for the dense fallback

UB = 24  # blocks per main streaming unit
FINAL_CHUNKS = [4, 2, 2]  # block split of the final unit (short drain)
FB = sum(FINAL_CHUNKS)  # 8
PREFETCH = 2  # units of DMA issued ahead of compute

_CACHED_NC = {}


def build_nc(nblocks):
    ncols = nblocks * 128
    nc = bacc.Bacc(
        "TRN2", target_bir_lowering=False, debug=False, num_devices=NCORES
    )
    u_d = nc.dram_tensor("u", [C, ncols], BF16, kind="ExternalInput")
    m_d = nc.dram_tensor("m", [C, ncols], BF16, kind="ExternalInput")
    # out[p, blk, s] = stat s (num/uu/mm) of packed position blk*128+p
    out_d = nc.dram_tensor("out", [128, nblocks * 3], F32, kind="ExternalOutput")

    # unit list: spans of blocks; final FB blocks split into small chunks
    spans = []
    blk = 0
    while blk < nblocks - FB:
        w = min(UB, nblocks - FB - blk)
        spans.append((blk, w))
        blk += w
    n_main = len(spans)
    for w in FINAL_CHUNKS:
        spans.append((blk, w))
        blk += w
    assert blk == nblocks

    with tile.TileContext(nc) as tc, ExitStack() as ctx:
        const_pool = ctx.enter_context(tc.tile_pool(name="const", bufs=1))
        in_pool = ctx.enter_context(tc.tile_pool(name="inp", bufs=3))
        tmp_pool = ctx.enter_context(tc.tile_pool(name="tmp", bufs=2))
        out_pool = ctx.enter_context(tc.tile_pool(name="outp", bufs=1))
        psum_pool = ctx.enter_context(
            tc.tile_pool(name="psum", bufs=1, space="PSUM")
        )

        ones_t = const_pool.tile([128, 1], BF16)
        nc.vector.memset(ones_t[:], 1.0)
        # single PSUM bank holds all stats: cols blk*3 + s
        P = psum_pool.tile([128, nblocks, 3], F32, name="P", tag="P")
        stats_t = out_pool.tile([128, nblocks, 3], F32)

        in_tiles = {}

        def issue_dma(i):
            blk0, w = spans[i]
            small = i >= n_main
            csl = slice(blk0 * 128, (blk0 + w) * 128)
            ums = []
            for ch in range(NCHUNK):
                row0 = ch * 128
                rsl = slice(row0, row0 + 128)
                if small:
                    u_t = in_pool.tile([128, 512], BF16, name=f"qu{ch}", tag=f"qu{ch}", bufs=2)
                    m_t = in_pool.tile([128, 512], BF16, name=f"qm{ch}", tag=f"qm{ch}", bufs=2)
                    u_t, m_t = u_t[:, : w * 128], m_t[:, : w * 128]
                else:
                    u_t = in_pool.tile([128, UB * 128], BF16, name=f"u{ch}", tag=f"u{ch}")
                    m_t = in_pool.tile([128, UB * 128], BF16, name=f"m{ch}", tag=f"m{ch}")
                    u_t, m_t = u_t[:, : w * 128], m_t[:, : w * 128]
                nc.sync.dma_start(u_t, u_d[rsl, csl])
                nc.sync.dma_start(m_t, m_d[rsl, csl])
                ums.append((u_t, m_t))
            in_tiles[i] = ums

        def flush(blk0, blk1, last=False):
            # PSUM has no DMA route: DVE copy to SBUF + DMA out. Mid-
            # stream flushes ride the idle SWDGE (gpsimd) queue so their
            # waits never block the input-DMA ring; the final flush uses
            # the (by then empty) sync ring for its lower latency.
            nc.vector.tensor_copy(
                stats_t[:, blk0:blk1, :], P[:, blk0:blk1, :]
            )
            eng = nc.sync if last else nc.gpsimd
            eng.dma_start(
                out_d[:, blk0 * 3 : blk1 * 3], stats_t[:, blk0:blk1, :]
            )

        for j in range(PREFETCH):
            issue_dma(j)

        mm_ctr = 0
        for i, (blk0, w) in enumerate(spans):
            if i + PREFETCH < len(spans):
                issue_dma(i + PREFETCH)
            ums = in_tiles.pop(i)
            small = i >= n_main
            wc = w * 128

            prods = []  # prods[ch] = (num, uu, mm)
            for ch, (u_t, m_t) in enumerate(ums):
                if small:
                    num_t = tmp_pool.tile([128, 512], BF16, name=f"qnum{ch}", tag=f"qnum{ch}", bufs=2
                    )[:, :wc]
                    uu_t = tmp_pool.tile([128, 512], BF16, name=f"quu{ch}", tag=f"quu{ch}", bufs=2
                    )[:, :wc]
                    mm_t = tmp_pool.tile([128, 512], BF16, name=f"qmm{ch}", tag=f"qmm{ch}", bufs=2
                    )[:, :wc]
                else:
                    num_t = tmp_pool.tile([128, UB * 128], BF16, name=f"num{ch}", tag=f"num{ch}"
                    )[:, :wc]
                    uu_t = tmp_pool.tile([128, UB * 128], BF16, name=f"uu{ch}", tag=f"uu{ch}"
                    )[:, :wc]
                    mm_t = tmp_pool.tile([128, UB * 128], BF16, name=f"mm{ch}", tag=f"mm{ch}"
                    )[:, :wc]
                nc.vector.tensor_mul(num_t, u_t, m_t)
                nc.scalar.square(uu_t, u_t)
                # bf16 gives DVE 2x throughput: it takes num and most m*m;
                # ACT keeps the u squares and the otherwise-idle GpSimd
                # absorbs every 4th m*m so no engine exceeds ~82% of the
                # DMA cadence
                nc.vector.tensor_mul(mm_t, m_t, m_t)
                mm_ctr += 1
                prods.append((num_t, uu_t, mm_t))

            for s in range(3):
                for pb in range(w):
                    for ch in range(NCHUNK):
                        nc.tensor.matmul(
                            P[:, blk0 + pb, s : s + 1],
                            prods[ch][s][:, pb * 128 : (pb + 1) * 128],
                            ones_t[:, :],
                            start=(ch == 0),
                            stop=(ch == NCHUNK - 1),
                        )

            # flush the previous unit's stats one unit after its last
            # matmul was emitted, so the copy never stalls DVE
            if 0 < i:
                pblk0, pw = spans[i - 1]
                flush(pblk0, pblk0 + pw)

        blk0, w = spans[-1]
        flush(blk0, blk0 + w, last=True)

    nc.compile()
    return nc


def get_nc(nblocks=CAPB):
    if nblocks not in _CACHED_NC:
        _CACHED_NC[nblocks] = build_nc(nblocks)
    return _CACHED_NC[nblocks]


def _pack_core(u4, m4, mask4, ncols):
    """Gather masked channel-columns of 4 batches into [C, ncols] bf16."""
    usegs, msegs = [], []
    for b in range(BL):
        idx = np.nonzero(mask4[b])[0]
        usegs.append(u4[b].reshape(C, HWX)[:, idx])
        msegs.append(m4[b].reshape(C, HWX)[:, idx])
    u_p = np.concatenate(usegs, axis=1)
    cnt = u_p.shape[1]
    up = np.zeros((C, ncols), dtype=ml_dtypes.bfloat16)
    mp = np.zeros((C, ncols), dtype=ml_dtypes.bfloat16)
    up[:, :cnt] = u_p.astype(ml_dtypes.bfloat16)
    mp[:, :cnt] = np.concatenate(msegs, axis=1).astype(ml_dtypes.bfloat16)
    return up, mp, cnt


def _run(unmasked, masked, latent_mask):
    mask = np.asarray(latent_mask) != 0
    mask_flat = mask.reshape(B, HWX)
    counts = [
        int(mask_flat[i * BL : (i + 1) * BL].sum()) for i in range(NCORES)
    ]

    if max(counts) <= CAP:
        nblocks = CAPB
        in_maps, valid = [], []
        for i in range(NCORES):
            sl = slice(i * BL, (i + 1) * BL)
            up, mp, cnt = _pack_core(
                unmasked[sl], masked[sl], mask_flat[sl], CAP
            )
            in_maps.append({"u": up, "m": mp})
            w = np.zeros(CAP, dtype=bool)
            w[:cnt] = True
            valid.append(w)
    else:
        # dense fallback: all positions, mask applied on host
        nblocks = DENSEB
        in_maps, valid = [], []
        for i in range(NCORES):
            sl = slice(i * BL, (i + 1) * BL)
            up = np.ascontiguousarray(
                np.asarray(unmasked[sl], dtype=np.float32).transpose(1, 0, 2, 3)
            ).reshape(C, NPOS).astype(ml_dtypes.bfloat16)
            mp = np.ascontiguousarray(
                np.asarray(masked[sl], dtype=np.float32).transpose(1, 0, 2, 3)
            ).reshape(C, NPOS).astype(ml_dtypes.bfloat16)
            in_maps.append({"u": up, "m": mp})
            valid.append(mask_flat[sl].reshape(NPOS))

    nc = get_nc(nblocks)
    return nc, in_maps, valid, float(mask.sum()), nblocks


def _finalize(results, valid, den, nblocks):
    num = 0.0
    for res, w in zip(results, valid):
        out = np.asarray(res["out"], dtype=np.float64).reshape(
            128, nblocks, 3
        )
        # out[p, blk, s] -> stats[s, blk*128+p]
        stats = out.transpose(2, 1, 0).reshape(3, nblocks * 128)
        n, uu, mm = stats[0][w], stats[1][w], stats[2][w]
        num += (n / np.sqrt(uu * mm)).sum()
    return np.float32(num / den)


def kernel(unmasked_latent_tensors, masked_latent_tensors, latent_mask, **kw):
    nc, in_maps, valid, den, nblocks = _run(
        np.asarray(unmasked_latent_tensors, dtype=np.float32),
        np.asarray(masked_latent_tensors, dtype=np.float32),
        np.asarray(latent_mask),
    )
    res = run_bass_kernel_spmd(nc, in_maps, list(range(NCORES)))
    return _finalize(res.results, valid, den, nblocks)


def kernel_traced(unmasked_latent_tensors, masked_latent_tensors, latent_mask):
    """Like kernel() but with NTFF tracing; returns (value, BassKernelResults)."""
    nc, in_maps, valid, den, nblocks = _run(
        np.asarray(unmasked_latent_tensors, dtype=np.float32),
        np.asarray(masked_latent_tensors, dtype=np.float32),
        np.asarray(latent_mask),
    )
    res = run_bass_kernel_spmd(nc, in_maps, list(range(NCORES)), trace=True)
    return _finalize(res.results, valid, den, nblocks), res


# revision 26
# speedup vs baseline: 1.1509x; 1.0473x over previous
"""Trainium2 Bass kernel for ContrastiveMaskedPatchSimilarity loss.

Computes: per-position cosine similarity along the channel axis of two
[32, 256, 64, 64] f32 tensors, then a masked mean -> scalar.

Strategy (pure data parallel over 8 NeuronCores, batch-sharded 4 each):
  - The masked mean only needs sim at mask==1 positions (~50%). The host
    gathers just those channel columns into a packed [256, 8704] array
    per core (zero-padded), halving the HBM traffic that is this
    memory-bound problem's entire roofline. A dense-layout NEFF is
    compiled lazily as a fallback if a mask ever exceeds the packed
    capacity.
  - The kernel streams the packed u/m and produces the three
    per-position channel sums (num=sum(u*m), uu=sum(u*u), mm=sum(m*m));
    the tiny nonlinear tail (sim=num/sqrt(uu*mm), masked mean) runs on
    host. The device side is a pure streaming pipeline with no
    mid-stream epilogue stalls.
  - Layout on chip: [channel-chunk (128) = partitions, position = free].
    All input DMAs on the sync/SP HWDGE ring: 8KB/partition descriptors
    saturate the 16 SDMA engines' ~27GiB/s line rate (~430GB/s).
  - Elementwise products (u*m, u*u, m*m) on DVE/ACT, written as bf16.
  - Channel reduction via TensorE: per position-block column, the two
    chunks' product slices [128ch x 128pos] are matmul'd against
    ones[128,1] back-to-back into the same PSUM slot (start/stop
    accumulation) -> no chunk-combining ops anywhere.
  - Stats stream out per unit (DVE copy PSUM->SBUF + SWDGE DMA, emitted
    one unit late); the final unit is processed in small chunks so the
    post-last-DMA drain is ~4us.
"""

import sys
from contextlib import ExitStack

import numpy as np

sys.path.insert(0, "/opt/trn_rl_repo")

import ml_dtypes  # noqa: E402
import concourse.bass as bass  # noqa: E402
import concourse.tile as tile  # noqa: E402
from concourse import bacc, mybir  # noqa: E402
from concourse.bass_utils import run_bass_kernel_spmd  # noqa: E402

B, C, H, W = 32, 256, 64, 64
NCORES = 8
BL = B // NCORES  # batches per core: 4
HWX = H * W  # 4096
NPOS = BL * HWX  # positions per core: 16384
NCHUNK = C // 128  # channel chunks: 2

F32 = mybir.dt.float32
BF16 = mybir.dt.bfloat16

# packed capacity: max masked positions per core the fast path handles.
# counts are ~binomial(16384, 0.5) (sigma 64); 8320 is mean + 2 sigma and
# 96 above the actual per-core max for the reference's fixed seed; any
# larger mask falls back to the dense-layout NEFF.
CAPB = 65  # capacity in 128-position blocks
CAP = CAPB * 128  # 8320
DENSEB = NPOS // 128  # 128 blocks for the dense fallback

UB = 24  # blocks per main streaming unit
FINAL_CHUNKS = [4, 2, 2]  # block split of the final unit (short drain)
FB = sum(FINAL_CHUNKS)  # 8
PREFETCH = 2  # units of DMA issued ahead of compute

_CACHED_NC = {}


def build_nc(nblocks):
    ncols = nblocks * 128
    nc = bacc.Bacc(
        "TRN2", target_bir_lowering=False, debug=False, num_devices=NCORES
    )
    u_d = nc.dram_tensor("u", [C, ncols], BF16, kind="ExternalInput")
    m_d = nc.dram_tensor("m", [C, ncols], BF16, kind="ExternalInput")
    # out[p, blk, s] = stat s (num/uu/mm) of packed position blk*128+p
    out_d = nc.dram_tensor("out", [128, nblocks * 3], F32, kind="ExternalOutput")

    # unit list: small first span (fast ramp), big middle spans, small
    # final chunks (short drain)
    widths = [4]
    rem = nblocks - 4 - FB
    while rem > 0:
        w = min(UB, rem)
        widths.append(w)
        rem -= w
    widths += FINAL_CHUNKS
    spans = []
    blk = 0
    for w in widths:
        spans.append((blk, w))
        blk += w
    assert blk == nblocks

    with tile.TileContext(nc) as tc, ExitStack() as ctx:
        const_pool = ctx.enter_context(tc.tile_pool(name="const", bufs=1))
        in_pool = ctx.enter_context(tc.tile_pool(name="inp", bufs=3))
        tmp_pool = ctx.enter_context(tc.tile_pool(name="tmp", bufs=2))
        out_pool = ctx.enter_context(tc.tile_pool(name="outp", bufs=1))
        psum_pool = ctx.enter_context(
            tc.tile_pool(name="psum", bufs=2, space="PSUM")
        )

        ones_t = const_pool.tile([128, 1], BF16)
        nc.vector.memset(ones_t[:], 1.0)
        stats_t = out_pool.tile([128, nblocks, 3], F32)

        in_tiles = {}

        def issue_dma(i):
            blk0, w = spans[i]
            csl = slice(blk0 * 128, (blk0 + w) * 128)
            ums = []
            for ch in range(NCHUNK):
                row0 = ch * 128
                rsl = slice(row0, row0 + 128)
                u_t = in_pool.tile([128, UB * 128], BF16, name=f"u{ch}", tag=f"u{ch}")
                m_t = in_pool.tile([128, UB * 128], BF16, name=f"m{ch}", tag=f"m{ch}")
                u_t, m_t = u_t[:, : w * 128], m_t[:, : w * 128]
                nc.sync.dma_start(u_t, u_d[rsl, csl])
                nc.sync.dma_start(m_t, m_d[rsl, csl])
                ums.append((u_t, m_t))
            in_tiles[i] = ums

        psums = {}

        def flush(i, last=False):
            # PSUM has no DMA route: DVE copy to SBUF + DMA out. Mid-
            # stream flushes ride the idle SWDGE (gpsimd) queue so their
            # waits never block the input-DMA ring; the final flush uses
            # the (by then empty) sync ring for its lower latency.
            blk0, w = spans[i]
            Pu = psums.pop(i)
            nc.vector.tensor_copy(
                stats_t[:, blk0 : blk0 + w, :], Pu[:, :w, :]
            )
            eng = nc.sync if last else nc.gpsimd
            eng.dma_start(
                out_d[:, blk0 * 3 : (blk0 + w) * 3],
                stats_t[:, blk0 : blk0 + w, :],
            )

        for j in range(PREFETCH):
            issue_dma(j)

        mm_ctr = 0
        for i, (blk0, w) in enumerate(spans):
            if i + PREFETCH < len(spans):
                issue_dma(i + PREFETCH)
            ums = in_tiles.pop(i)
            wc = w * 128
            # per-unit PSUM tile (rotating banks): flush copies of unit
            # i-1 and matmuls of unit i touch different tiles, so no
            # false PSUM dependencies are possible
            Pu = psum_pool.tile([128, UB, 3], F32, name="P", tag="P")
            psums[i] = Pu

            prods = []  # prods[ch] = (num, uu, mm)
            for ch, (u_t, m_t) in enumerate(ums):
                num_t = tmp_pool.tile([128, UB * 128], BF16, name=f"num{ch}", tag=f"num{ch}"
                )[:, :wc]
                uu_t = tmp_pool.tile([128, UB * 128], BF16, name=f"uu{ch}", tag=f"uu{ch}"
                )[:, :wc]
                mm_t = tmp_pool.tile([128, UB * 128], BF16, name=f"mm{ch}", tag=f"mm{ch}"
                )[:, :wc]
                nc.vector.tensor_mul(num_t, u_t, m_t)
                nc.scalar.square(uu_t, u_t)
                # bf16 gives DVE 2x throughput: it takes num and most m*m;
                # ACT keeps the u squares and the otherwise-idle GpSimd
                # absorbs every 4th m*m so no engine exceeds ~82% of the
                # DMA cadence
                nc.vector.tensor_mul(mm_t, m_t, m_t)
                mm_ctr += 1
                prods.append((num_t, uu_t, mm_t))

            for s in range(3):
                for pb in range(w):
                    for ch in range(NCHUNK):
                        nc.tensor.matmul(
                            Pu[:, pb, s : s + 1],
                            prods[ch][s][:, pb * 128 : (pb + 1) * 128],
                            ones_t[:, :],
                            start=(ch == 0),
                            stop=(ch == NCHUNK - 1),
                        )

            # flush the previous unit's stats one unit after its last
            # matmul was emitted, so the copy never stalls DVE
            if 0 < i:
                flush(i - 1)

        flush(len(spans) - 1, last=True)

    nc.compile()
    return nc


def get_nc(nblocks=CAPB):
    if nblocks not in _CACHED_NC:
        _CACHED_NC[nblocks] = build_nc(nblocks)
    return _CACHED_NC[nblocks]


def _pack_core(u4, m4, mask4, ncols):
    """Gather masked channel-columns of 4 batches into [C, ncols] bf16."""
    usegs, msegs = [], []
    for b in range(BL):
        idx = np.nonzero(mask4[b])[0]
        usegs.append(u4[b].reshape(C, HWX)[:, idx])
        msegs.append(m4[b].reshape(C, HWX)[:, idx])
    u_p = np.concatenate(usegs, axis=1)
    cnt = u_p.shape[1]
    up = np.zeros((C, ncols), dtype=ml_dtypes.bfloat16)
    mp = np.zeros((C, ncols), dtype=ml_dtypes.bfloat16)
    up[:, :cnt] = u_p.astype(ml_dtypes.bfloat16)
    mp[:, :cnt] = np.concatenate(msegs, axis=1).astype(ml_dtypes.bfloat16)
    return up, mp, cnt


def _run(unmasked, masked, latent_mask):
    mask = np.asarray(latent_mask) != 0
    mask_flat = mask.reshape(B, HWX)
    counts = [
        int(mask_flat[i * BL : (i + 1) * BL].sum()) for i in range(NCORES)
    ]

    if max(counts) <= CAP:
        nblocks = CAPB
        in_maps, valid = [], []
        for i in range(NCORES):
            sl = slice(i * BL, (i + 1) * BL)
            up, mp, cnt = _pack_core(
                unmasked[sl], masked[sl], mask_flat[sl], CAP
            )
            in_maps.append({"u": up, "m": mp})
            w = np.zeros(CAP, dtype=bool)
            w[:cnt] = True
            valid.append(w)
    else:
        # dense fallback: all positions, mask applied on host
        nblocks = DENSEB
        in_maps, valid = [], []
        for i in range(NCORES):
            sl = slice(i * BL, (i + 1) * BL)
            up = np.ascontiguousarray(
                np.asarray(unmasked[sl], dtype=np.float32).transpose(1, 0, 2, 3)
            ).reshape(C, NPOS).astype(ml_dtypes.bfloat16)
            mp = np.ascontiguousarray(
                np.asarray(masked[sl], dtype=np.float32).transpose(1, 0, 2, 3)
            ).reshape(C, NPOS).astype(ml_dtypes.bfloat16)
            in_maps.append({"u": up, "m": mp})
            valid.append(mask_flat[sl].reshape(NPOS))

    nc = get_nc(nblocks)
    return nc, in_maps, valid, float(mask.sum()), nblocks


def _finalize(results, valid, den, nblocks):
    num = 0.0
    for res, w in zip(results, valid):
        out = np.asarray(res["out"], dtype=np.float64).reshape(
            128, nblocks, 3
        )
        # out[p, blk, s] -> stats[s, blk*128+p]
        stats = out.transpose(2, 1, 0).reshape(3, nblocks * 128)
        n, uu, mm = stats[0][w], stats[1][w], stats[2][w]
        num += (n / np.sqrt(uu * mm)).sum()
    return np.float32(num / den)


def kernel(unmasked_latent_tensors, masked_latent_tensors, latent_mask, **kw):
    nc, in_maps, valid, den, nblocks = _run(
        np.asarray(unmasked_latent_tensors, dtype=np.float32),
        np.asarray(masked_latent_tensors, dtype=np.float32),
        np.asarray(latent_mask),
    )
    res = run_bass_kernel_spmd(nc, in_maps, list(range(NCORES)))
    return _finalize(res.results, valid, den, nblocks)


def kernel_traced(unmasked_latent_tensors, masked_latent_tensors, latent_mask):
    """Like kernel() but with NTFF tracing; returns (value, BassKernelResults)."""
    nc, in_maps, valid, den, nblocks = _run(
        np.asarray(unmasked_latent_tensors, dtype=np.float32),
        np.asarray(masked_latent_tensors, dtype=np.float32),
        np.asarray(latent_mask),
    )
    res = run_bass_kernel_spmd(nc, in_maps, list(range(NCORES)), trace=True)
    return _finalize(res.results, valid, den, nblocks), res


# revision 27
# speedup vs baseline: 1.1746x; 1.0206x over previous
"""Trainium2 Bass kernel for ContrastiveMaskedPatchSimilarity loss.

Computes: per-position cosine similarity along the channel axis of two
[32, 256, 64, 64] f32 tensors, then a masked mean -> scalar.

Strategy (pure data parallel over 8 NeuronCores, batch-sharded 4 each):
  - The masked mean only needs sim at mask==1 positions (~50%). The host
    gathers just those channel columns into a packed [256, 8704] array
    per core (zero-padded), halving the HBM traffic that is this
    memory-bound problem's entire roofline. A dense-layout NEFF is
    compiled lazily as a fallback if a mask ever exceeds the packed
    capacity.
  - The kernel streams the packed u/m and produces the three
    per-position channel sums (num=sum(u*m), uu=sum(u*u), mm=sum(m*m));
    the tiny nonlinear tail (sim=num/sqrt(uu*mm), masked mean) runs on
    host. The device side is a pure streaming pipeline with no
    mid-stream epilogue stalls.
  - Layout on chip: [channel-chunk (128) = partitions, position = free].
    All input DMAs on the sync/SP HWDGE ring: 8KB/partition descriptors
    saturate the 16 SDMA engines' ~27GiB/s line rate (~430GB/s).
  - Elementwise products (u*m, u*u, m*m) on DVE/ACT, written as bf16.
  - Channel reduction via TensorE: per position-block column, the two
    chunks' product slices [128ch x 128pos] are matmul'd against
    ones[128,1] back-to-back into the same PSUM slot (start/stop
    accumulation) -> no chunk-combining ops anywhere.
  - Stats stream out per unit (DVE copy PSUM->SBUF + SWDGE DMA, emitted
    one unit late); the final unit is processed in small chunks so the
    post-last-DMA drain is ~4us.
"""

import sys
from contextlib import ExitStack

import numpy as np

sys.path.insert(0, "/opt/trn_rl_repo")

import ml_dtypes  # noqa: E402
import concourse.bass as bass  # noqa: E402
import concourse.tile as tile  # noqa: E402
from concourse import bacc, mybir  # noqa: E402
from concourse.bass_utils import run_bass_kernel_spmd  # noqa: E402

B, C, H, W = 32, 256, 64, 64
NCORES = 8
BL = B // NCORES  # batches per core: 4
HWX = H * W  # 4096
NPOS = BL * HWX  # positions per core: 16384
NCHUNK = C // 128  # channel chunks: 2

F32 = mybir.dt.float32
BF16 = mybir.dt.bfloat16

# packed capacity: max masked positions per core the fast path handles.
# counts are ~binomial(16384, 0.5) (sigma 64); 8320 is mean + 2 sigma and
# 96 above the actual per-core max for the reference's fixed seed; any
# larger mask falls back to the dense-layout NEFF.
CAPB = 65  # capacity in 128-position blocks
CAP = CAPB * 128  # 8320
DENSEB = NPOS // 128  # 128 blocks for the dense fallback

UB = 24  # blocks per main streaming unit
FINAL_CHUNKS = [9, 4]  # block split of the final unit (short drain)
FB = sum(FINAL_CHUNKS)  # 13
PREFETCH = 2  # units of DMA issued ahead of compute

_CACHED_NC = {}


def build_nc(nblocks):
    ncols = nblocks * 128
    nc = bacc.Bacc(
        "TRN2", target_bir_lowering=False, debug=False, num_devices=NCORES
    )
    u_d = nc.dram_tensor("u", [C, ncols], BF16, kind="ExternalInput")
    m_d = nc.dram_tensor("m", [C, ncols], BF16, kind="ExternalInput")
    # out[p, blk, s] = stat s (num/uu/mm) of packed position blk*128+p
    out_d = nc.dram_tensor("out", [128, nblocks * 3], F32, kind="ExternalOutput")

    # unit list: small first span (fast ramp), big middle spans, small
    # final chunks (short drain)
    widths = [4]
    rem = nblocks - 4 - FB
    while rem > 0:
        w = min(UB, rem)
        widths.append(w)
        rem -= w
    widths += FINAL_CHUNKS
    spans = []
    blk = 0
    for w in widths:
        spans.append((blk, w))
        blk += w
    assert blk == nblocks

    with tile.TileContext(nc) as tc, ExitStack() as ctx:
        const_pool = ctx.enter_context(tc.tile_pool(name="const", bufs=1))
        in_pool = ctx.enter_context(tc.tile_pool(name="inp", bufs=4))
        tmp_pool = ctx.enter_context(tc.tile_pool(name="tmp", bufs=2))
        out_pool = ctx.enter_context(tc.tile_pool(name="outp", bufs=1))
        psum_pool = ctx.enter_context(
            tc.tile_pool(name="psum", bufs=2, space="PSUM")
        )

        ones_t = const_pool.tile([128, 1], BF16)
        nc.vector.memset(ones_t[:], 1.0)
        stats_t = out_pool.tile([128, nblocks, 3], F32)

        in_tiles = {}

        def issue_dma(i):
            blk0, w = spans[i]
            csl = slice(blk0 * 128, (blk0 + w) * 128)
            ums = []
            for ch in range(NCHUNK):
                row0 = ch * 128
                rsl = slice(row0, row0 + 128)
                u_t = in_pool.tile([128, UB * 128], BF16, name=f"u{ch}", tag=f"u{ch}")
                m_t = in_pool.tile([128, UB * 128], BF16, name=f"m{ch}", tag=f"m{ch}")
                u_t, m_t = u_t[:, : w * 128], m_t[:, : w * 128]
                nc.sync.dma_start(u_t, u_d[rsl, csl])
                nc.sync.dma_start(m_t, m_d[rsl, csl])
                ums.append((u_t, m_t))
            in_tiles[i] = ums

        psums = {}

        def flush(i, last=False):
            # PSUM has no DMA route: DVE copy to SBUF + DMA out. Mid-
            # stream flushes ride the idle SWDGE (gpsimd) queue so their
            # waits never block the input-DMA ring; the final flush uses
            # the (by then empty) sync ring for its lower latency.
            blk0, w = spans[i]
            Pu = psums.pop(i)
            nc.vector.tensor_copy(
                stats_t[:, blk0 : blk0 + w, :], Pu[:, :w, :]
            )
            eng = nc.sync if last else nc.gpsimd
            eng.dma_start(
                out_d[:, blk0 * 3 : (blk0 + w) * 3],
                stats_t[:, blk0 : blk0 + w, :],
            )

        for j in range(PREFETCH):
            issue_dma(j)

        mm_ctr = 0
        for i, (blk0, w) in enumerate(spans):
            if i + PREFETCH < len(spans):
                issue_dma(i + PREFETCH)
            ums = in_tiles.pop(i)
            wc = w * 128
            # per-unit PSUM tile (rotating banks): flush copies of unit
            # i-1 and matmuls of unit i touch different tiles, so no
            # false PSUM dependencies are possible
            Pu = psum_pool.tile([128, UB, 3], F32, name="P", tag="P")
            psums[i] = Pu

            prods = []  # prods[ch] = (num, uu, mm)
            for ch, (u_t, m_t) in enumerate(ums):
                num_t = tmp_pool.tile([128, UB * 128], BF16, name=f"num{ch}", tag=f"num{ch}"
                )[:, :wc]
                uu_t = tmp_pool.tile([128, UB * 128], BF16, name=f"uu{ch}", tag=f"uu{ch}"
                )[:, :wc]
                mm_t = tmp_pool.tile([128, UB * 128], BF16, name=f"mm{ch}", tag=f"mm{ch}"
                )[:, :wc]
                nc.vector.tensor_mul(num_t, u_t, m_t)
                nc.scalar.square(uu_t, u_t)
                # bf16 gives DVE 2x throughput: it takes num and most m*m;
                # ACT keeps the u squares and the otherwise-idle GpSimd
                # absorbs every 4th m*m so no engine exceeds ~82% of the
                # DMA cadence
                nc.vector.tensor_mul(mm_t, m_t, m_t)
                mm_ctr += 1
                prods.append((num_t, uu_t, mm_t))

            for s in range(3):
                for pb in range(w):
                    for ch in range(NCHUNK):
                        nc.tensor.matmul(
                            Pu[:, pb, s : s + 1],
                            prods[ch][s][:, pb * 128 : (pb + 1) * 128],
                            ones_t[:, :],
                            start=(ch == 0),
                            stop=(ch == NCHUNK - 1),
                        )

            # flush the previous unit's stats one unit after its last
            # matmul was emitted, so the copy never stalls DVE
            if 0 < i:
                flush(i - 1)

        flush(len(spans) - 1, last=True)

    nc.compile()
    return nc


def get_nc(nblocks=CAPB):
    if nblocks not in _CACHED_NC:
        _CACHED_NC[nblocks] = build_nc(nblocks)
    return _CACHED_NC[nblocks]


def _pack_core(u4, m4, mask4, ncols):
    """Gather masked channel-columns of 4 batches into [C, ncols] bf16."""
    usegs, msegs = [], []
    for b in range(BL):
        idx = np.nonzero(mask4[b])[0]
        usegs.append(u4[b].reshape(C, HWX)[:, idx])
        msegs.append(m4[b].reshape(C, HWX)[:, idx])
    u_p = np.concatenate(usegs, axis=1)
    cnt = u_p.shape[1]
    up = np.zeros((C, ncols), dtype=ml_dtypes.bfloat16)
    mp = np.zeros((C, ncols), dtype=ml_dtypes.bfloat16)
    up[:, :cnt] = u_p.astype(ml_dtypes.bfloat16)
    mp[:, :cnt] = np.concatenate(msegs, axis=1).astype(ml_dtypes.bfloat16)
    return up, mp, cnt


def _run(unmasked, masked, latent_mask):
    mask = np.asarray(latent_mask) != 0
    mask_flat = mask.reshape(B, HWX)
    counts = [
        int(mask_flat[i * BL : (i + 1) * BL].sum()) for i in range(NCORES)
    ]

    if max(counts) <= CAP:
        nblocks = CAPB
        in_maps, valid = [], []
        for i in range(NCORES):
            sl = slice(i * BL, (i + 1) * BL)
            up, mp, cnt = _pack_core(
                unmasked[sl], masked[sl], mask_flat[sl], CAP
            )
            in_maps.append({"u": up, "m": mp})
            w = np.zeros(CAP, dtype=bool)
            w[:cnt] = True
            valid.append(w)
    else:
        # dense fallback: all positions, mask applied on host
        nblocks = DENSEB
        in_maps, valid = [], []
        for i in range(NCORES):
            sl = slice(i * BL, (i + 1) * BL)
            up = np.ascontiguousarray(
                np.asarray(unmasked[sl], dtype=np.float32).transpose(1, 0, 2, 3)
            ).reshape(C, NPOS).astype(ml_dtypes.bfloat16)
            mp = np.ascontiguousarray(
                np.asarray(masked[sl], dtype=np.float32).transpose(1, 0, 2, 3)
            ).reshape(C, NPOS).astype(ml_dtypes.bfloat16)
            in_maps.append({"u": up, "m": mp})
            valid.append(mask_flat[sl].reshape(NPOS))

    nc = get_nc(nblocks)
    return nc, in_maps, valid, float(mask.sum()), nblocks


def _finalize(results, valid, den, nblocks):
    num = 0.0
    for res, w in zip(results, valid):
        out = np.asarray(res["out"], dtype=np.float64).reshape(
            128, nblocks, 3
        )
        # out[p, blk, s] -> stats[s, blk*128+p]
        stats = out.transpose(2, 1, 0).reshape(3, nblocks * 128)
        n, uu, mm = stats[0][w], stats[1][w], stats[2][w]
        num += (n / np.sqrt(uu * mm)).sum()
    return np.float32(num / den)


def kernel(unmasked_latent_tensors, masked_latent_tensors, latent_mask, **kw):
    nc, in_maps, valid, den, nblocks = _run(
        np.asarray(unmasked_latent_tensors, dtype=np.float32),
        np.asarray(masked_latent_tensors, dtype=np.float32),
        np.asarray(latent_mask),
    )
    res = run_bass_kernel_spmd(nc, in_maps, list(range(NCORES)))
    return _finalize(res.results, valid, den, nblocks)


def kernel_traced(unmasked_latent_tensors, masked_latent_tensors, latent_mask):
    """Like kernel() but with NTFF tracing; returns (value, BassKernelResults)."""
    nc, in_maps, valid, den, nblocks = _run(
        np.asarray(unmasked_latent_tensors, dtype=np.float32),
        np.asarray(masked_latent_tensors, dtype=np.float32),
        np.asarray(latent_mask),
    )
    res = run_bass_kernel_spmd(nc, in_maps, list(range(NCORES)), trace=True)
    return _finalize(res.results, valid, den, nblocks), res


# revision 28
# speedup vs baseline: 1.2077x; 1.0282x over previous
"""Trainium2 Bass kernel for ContrastiveMaskedPatchSimilarity loss.

Computes: per-position cosine similarity along the channel axis of two
[32, 256, 64, 64] f32 tensors, then a masked mean -> scalar.

Strategy (pure data parallel over 8 NeuronCores, batch-sharded 4 each):
  - The masked mean only needs sim at mask==1 positions (~50%). The host
    gathers just those channel columns into a packed [256, 8704] array
    per core (zero-padded), halving the HBM traffic that is this
    memory-bound problem's entire roofline. A dense-layout NEFF is
    compiled lazily as a fallback if a mask ever exceeds the packed
    capacity.
  - The kernel streams the packed u/m and produces the three
    per-position channel sums (num=sum(u*m), uu=sum(u*u), mm=sum(m*m));
    the tiny nonlinear tail (sim=num/sqrt(uu*mm), masked mean) runs on
    host. The device side is a pure streaming pipeline with no
    mid-stream epilogue stalls.
  - Layout on chip: [channel-chunk (128) = partitions, position = free].
    All input DMAs on the sync/SP HWDGE ring: 8KB/partition descriptors
    saturate the 16 SDMA engines' ~27GiB/s line rate (~430GB/s).
  - Elementwise products (u*m, u*u, m*m) on DVE/ACT, written as bf16.
  - Channel reduction via TensorE: per position-block column, the two
    chunks' product slices [128ch x 128pos] are matmul'd against
    ones[128,1] back-to-back into the same PSUM slot (start/stop
    accumulation) -> no chunk-combining ops anywhere.
  - Stats stream out per unit (DVE copy PSUM->SBUF + SWDGE DMA, emitted
    one unit late); the final unit is processed in small chunks so the
    post-last-DMA drain is ~4us.
"""

import sys
from contextlib import ExitStack

import numpy as np

sys.path.insert(0, "/opt/trn_rl_repo")

import ml_dtypes  # noqa: E402
import concourse.bass as bass  # noqa: E402
import concourse.tile as tile  # noqa: E402
from concourse import bacc, mybir  # noqa: E402
from concourse.bass_utils import run_bass_kernel_spmd  # noqa: E402

B, C, H, W = 32, 256, 64, 64
NCORES = 8
BL = B // NCORES  # batches per core: 4
HWX = H * W  # 4096
NPOS = BL * HWX  # positions per core: 16384
NCHUNK = C // 128  # channel chunks: 2

F32 = mybir.dt.float32
BF16 = mybir.dt.bfloat16

# packed capacity: max masked positions per core the fast path handles.
# counts are ~binomial(16384, 0.5) (sigma 64); 8320 is mean + 2 sigma and
# 96 above the actual per-core max for the reference's fixed seed; any
# larger mask falls back to the dense-layout NEFF.
CAPB = 65  # capacity in 128-position blocks
CAP = CAPB * 128  # 8320
DENSEB = NPOS // 128  # 128 blocks for the dense fallback

UB = 32  # blocks per main streaming unit
FINAL_CHUNKS = [9, 4]  # block split of the final unit (short drain)
FB = sum(FINAL_CHUNKS)  # 13
PREFETCH = 2  # units of DMA issued ahead of compute

_CACHED_NC = {}


def build_nc(nblocks):
    ncols = nblocks * 128
    nc = bacc.Bacc(
        "TRN2", target_bir_lowering=False, debug=False, num_devices=NCORES
    )
    u_d = nc.dram_tensor("u", [C, ncols], BF16, kind="ExternalInput")
    m_d = nc.dram_tensor("m", [C, ncols], BF16, kind="ExternalInput")
    # out[p, blk, s] = stat s (num/uu/mm) of packed position blk*128+p
    out_d = nc.dram_tensor("out", [128, nblocks * 3], F32, kind="ExternalOutput")

    # unit list: small first span (fast ramp), big middle spans, small
    # final chunks (short drain)
    widths = [4]
    rem = nblocks - 4 - FB
    while rem > 0:
        w = min(UB, rem)
        widths.append(w)
        rem -= w
    widths += FINAL_CHUNKS
    spans = []
    blk = 0
    for w in widths:
        spans.append((blk, w))
        blk += w
    assert blk == nblocks

    with tile.TileContext(nc) as tc, ExitStack() as ctx:
        const_pool = ctx.enter_context(tc.tile_pool(name="const", bufs=1))
        in_pool = ctx.enter_context(tc.tile_pool(name="inp", bufs=3))
        tmp_pool = ctx.enter_context(tc.tile_pool(name="tmp", bufs=2))
        out_pool = ctx.enter_context(tc.tile_pool(name="outp", bufs=1))
        psum_pool = ctx.enter_context(
            tc.tile_pool(name="psum", bufs=2, space="PSUM")
        )

        ones_t = const_pool.tile([128, 1], BF16)
        nc.vector.memset(ones_t[:], 1.0)
        stats_t = out_pool.tile([128, nblocks, 3], F32)

        in_tiles = {}

        def issue_dma(i):
            blk0, w = spans[i]
            csl = slice(blk0 * 128, (blk0 + w) * 128)
            ums = []
            for ch in range(NCHUNK):
                row0 = ch * 128
                rsl = slice(row0, row0 + 128)
                u_t = in_pool.tile([128, UB * 128], BF16, name=f"u{ch}", tag=f"u{ch}")
                m_t = in_pool.tile([128, UB * 128], BF16, name=f"m{ch}", tag=f"m{ch}")
                u_t, m_t = u_t[:, : w * 128], m_t[:, : w * 128]
                nc.sync.dma_start(u_t, u_d[rsl, csl])
                nc.sync.dma_start(m_t, m_d[rsl, csl])
                ums.append((u_t, m_t))
            in_tiles[i] = ums

        psums = {}

        def flush(i, last=False):
            # PSUM has no DMA route: DVE copy to SBUF + DMA out. Mid-
            # stream flushes ride the idle SWDGE (gpsimd) queue so their
            # waits never block the input-DMA ring; the final flush uses
            # the (by then empty) sync ring for its lower latency.
            blk0, w = spans[i]
            Pu = psums.pop(i)
            nc.vector.tensor_copy(
                stats_t[:, blk0 : blk0 + w, :], Pu[:, :w, :]
            )
            eng = nc.sync if last else nc.gpsimd
            eng.dma_start(
                out_d[:, blk0 * 3 : (blk0 + w) * 3],
                stats_t[:, blk0 : blk0 + w, :],
            )

        for j in range(PREFETCH):
            issue_dma(j)

        mm_ctr = 0
        for i, (blk0, w) in enumerate(spans):
            if i + PREFETCH < len(spans):
                issue_dma(i + PREFETCH)
            ums = in_tiles.pop(i)
            wc = w * 128
            # per-unit PSUM tile (rotating banks): flush copies of unit
            # i-1 and matmuls of unit i touch different tiles, so no
            # false PSUM dependencies are possible
            Pu = psum_pool.tile([128, UB, 3], F32, name="P", tag="P")
            psums[i] = Pu

            prods = []  # prods[ch] = (num, uu, mm)
            for ch, (u_t, m_t) in enumerate(ums):
                num_t = tmp_pool.tile([128, UB * 128], BF16, name=f"num{ch}", tag=f"num{ch}"
                )[:, :wc]
                uu_t = tmp_pool.tile([128, UB * 128], BF16, name=f"uu{ch}", tag=f"uu{ch}"
                )[:, :wc]
                mm_t = tmp_pool.tile([128, UB * 128], BF16, name=f"mm{ch}", tag=f"mm{ch}"
                )[:, :wc]
                nc.vector.tensor_mul(num_t, u_t, m_t)
                nc.scalar.square(uu_t, u_t)
                # bf16 gives DVE 2x throughput: it takes num and most m*m;
                # ACT keeps the u squares and the otherwise-idle GpSimd
                # absorbs every 4th m*m so no engine exceeds ~82% of the
                # DMA cadence
                nc.vector.tensor_mul(mm_t, m_t, m_t)
                mm_ctr += 1
                prods.append((num_t, uu_t, mm_t))

            for s in range(3):
                for pb in range(w):
                    for ch in range(NCHUNK):
                        nc.tensor.matmul(
                            Pu[:, pb, s : s + 1],
                            prods[ch][s][:, pb * 128 : (pb + 1) * 128],
                            ones_t[:, :],
                            start=(ch == 0),
                            stop=(ch == NCHUNK - 1),
                        )

            # flush the previous unit's stats one unit after its last
            # matmul was emitted, so the copy never stalls DVE
            if 0 < i:
                flush(i - 1)

        flush(len(spans) - 1, last=True)

    nc.compile()
    return nc


def get_nc(nblocks=CAPB):
    if nblocks not in _CACHED_NC:
        _CACHED_NC[nblocks] = build_nc(nblocks)
    return _CACHED_NC[nblocks]


def _pack_core(u4, m4, mask4, ncols):
    """Gather masked channel-columns of 4 batches into [C, ncols] bf16."""
    usegs, msegs = [], []
    for b in range(BL):
        idx = np.nonzero(mask4[b])[0]
        usegs.append(u4[b].reshape(C, HWX)[:, idx])
        msegs.append(m4[b].reshape(C, HWX)[:, idx])
    u_p = np.concatenate(usegs, axis=1)
    cnt = u_p.shape[1]
    up = np.zeros((C, ncols), dtype=ml_dtypes.bfloat16)
    mp = np.zeros((C, ncols), dtype=ml_dtypes.bfloat16)
    up[:, :cnt] = u_p.astype(ml_dtypes.bfloat16)
    mp[:, :cnt] = np.concatenate(msegs, axis=1).astype(ml_dtypes.bfloat16)
    return up, mp, cnt


def _run(unmasked, masked, latent_mask):
    mask = np.asarray(latent_mask) != 0
    mask_flat = mask.reshape(B, HWX)
    counts = [
        int(mask_flat[i * BL : (i + 1) * BL].sum()) for i in range(NCORES)
    ]

    if max(counts) <= CAP:
        nblocks = CAPB
        in_maps, valid = [], []
        for i in range(NCORES):
            sl = slice(i * BL, (i + 1) * BL)
            up, mp, cnt = _pack_core(
                unmasked[sl], masked[sl], mask_flat[sl], CAP
            )
            in_maps.append({"u": up, "m": mp})
            w = np.zeros(CAP, dtype=bool)
            w[:cnt] = True
            valid.append(w)
    else:
        # dense fallback: all positions, mask applied on host
        nblocks = DENSEB
        in_maps, valid = [], []
        for i in range(NCORES):
            sl = slice(i * BL, (i + 1) * BL)
            up = np.ascontiguousarray(
                np.asarray(unmasked[sl], dtype=np.float32).transpose(1, 0, 2, 3)
            ).reshape(C, NPOS).astype(ml_dtypes.bfloat16)
            mp = np.ascontiguousarray(
                np.asarray(masked[sl], dtype=np.float32).transpose(1, 0, 2, 3)
            ).reshape(C, NPOS).astype(ml_dtypes.bfloat16)
            in_maps.append({"u": up, "m": mp})
            valid.append(mask_flat[sl].reshape(NPOS))

    nc = get_nc(nblocks)
    return nc, in_maps, valid, float(mask.sum()), nblocks


def _finalize(results, valid, den, nblocks):
    num = 0.0
    for res, w in zip(results, valid):
        out = np.asarray(res["out"], dtype=np.float64).reshape(
            128, nblocks, 3
        )
        # out[p, blk, s] -> stats[s, blk*128+p]
        stats = out.transpose(2, 1, 0).reshape(3, nblocks * 128)
        n, uu, mm = stats[0][w], stats[1][w], stats[2][w]
        num += (n / np.sqrt(uu * mm)).sum()
    return np.float32(num / den)


def kernel(unmasked_latent_tensors, masked_latent_tensors, latent_mask, **kw):
    nc, in_maps, valid, den, nblocks = _run(
        np.asarray(unmasked_latent_tensors, dtype=np.float32),
        np.asarray(masked_latent_tensors, dtype=np.float32),
        np.asarray(latent_mask),
    )
    res = run_bass_kernel_spmd(nc, in_maps, list(range(NCORES)))
    return _finalize(res.results, valid, den, nblocks)


def kernel_traced(unmasked_latent_tensors, masked_latent_tensors, latent_mask):
    """Like kernel() but with NTFF tracing; returns (value, BassKernelResults)."""
    nc, in_maps, valid, den, nblocks = _run(
        np.asarray(unmasked_latent_tensors, dtype=np.float32),
        np.asarray(masked_latent_tensors, dtype=np.float32),
        np.asarray(latent_mask),
    )
    res = run_bass_kernel_spmd(nc, in_maps, list(range(NCORES)), trace=True)
    return _finalize(res.results, valid, den, nblocks), res
